# revision 17
# baseline (speedup 1.0000x reference)
"""Bass/Trainium2 kernel for batched RBF (ScaleKernel(RBFKernel)) gram matrix.

Reference computation, for pos_embed [B, D, L] (B=4, D=64, L=4096):
    x  = pos_embed^T @ W^T + b          (per batch, feature-dim linear)
    xs = x / lengthscale
    out[b, q, k] = outputscale * exp(-0.5 * max(||xs_q - xs_k||^2, 0))

Sharding: 8 cores; core c -> batch b = c // 2, query half h = c % 2
(2048 queries each). Keys replicated per batch. Output [4, 4096, 4096] f32.

The tiny feature-dim linear + row norms (0.03% of FLOPs) are folded into
input preparation on the host; each core receives one packed operand tensor
with augmented contraction rows so a single matmul group per output tile
produces the full RBF log-kernel:

    G''[q, k] = xs_q . xs_k - 0.5||xs_q||^2 - 0.5||xs_k||^2 + ln(outputscale)

The fp32 operands are split into bf16 hi/lo pairs (x = hi + lo) and the
gram matrix computed as hi.hi + hi.lo + lo.hi accumulated in fp32 PSUM
(the dropped lo.lo term is ~6e-5 of the exponent). bf16 matmuls stream at
1 cycle/row vs fp32's 4, moving the kernel from PE-bound to DMA-bound.
The -0.5||x||^2 rows ride along as extra contraction rows paired with
exact-in-bf16 ones rows, plus second-residual rows 66/67 so their
precision is ~2^-27 relative. Device work per core: 384 matmuls, 8.4M exp
on ScalarE, 32 MB output DMA. The gpytorch clamp (out <= outputscale) is
applied on the host during unsharding: exp is monotone and outputscale>0,
so min(os*exp(t), os) == os*exp(-0.5*max(sq_dist,0)).
"""

import numpy as np
import ml_dtypes

import concourse.bass as bass
import concourse.tile as tile
from concourse import mybir
from concourse.bass_utils import run_bass_kernel_spmd

B, D, L = 4, 64, 4096
NCORES = 8
QH = L // 2     # queries per core
KQ = D + 4      # contraction rows (xs + 2 norm rows + 2 residual rows)
NB = 512        # one PSUM bank of fp32
OPS_W = L + QH  # packed operand width: [keys | queries]

F32 = mybir.dt.float32
BF16 = mybir.dt.bfloat16
AF = mybir.ActivationFunctionType
ALU = mybir.AluOpType

_prog_cache = {}


def _build_program():
    if "nc" in _prog_cache:
        return _prog_cache["nc"]

    nc = bass.Bass("TRN2")

    # [hi | lo] packed side by side -> one input DMA -> one sync wait
    ops = nc.dram_tensor("ops", [KQ, 2 * OPS_W], BF16, kind="ExternalInput").ap()
    out = nc.dram_tensor("out", [QH, L], F32, kind="ExternalOutput").ap()

    # raw-bass pipeline with manual semaphores: standalone wait_ge
    # instructions keep every compute instruction within walrus's
    # one-sync-wait-per-instruction codegen limit
    ops_sb = nc.alloc_sbuf_tensor("ops_sb", [KQ, 2 * OPS_W], BF16).ap()
    ots = [
        nc.alloc_sbuf_tensor(f"ot{i}", [128, L], F32).ap() for i in range(3)
    ]
    banks = [
        nc.alloc_psum_tensor(f"pb{i}", [128, NB], F32).ap() for i in range(8)
    ]

    HI, LO = 0, OPS_W
    NQ = QH // 128  # 16 query tiles
    NJ = L // NB    # 8 key chunks per tile

    with (
        nc.semaphore("dma_in") as dma_in,
        nc.semaphore("pe_sem") as pe_sem,
        nc.semaphore("act_sem") as act_sem,
        nc.semaphore("out_sem") as out_sem,
        nc.Block() as block,
    ):

        @block.sync
        def _(sync: bass.BassEngine):
            sync.dma_start(out=ops_sb[:], in_=ops[:]).then_inc(dma_in, 16)
            for qt in range(NQ):
                sync.wait_ge(act_sem, (qt + 1) * NJ)
                if qt:
                    # order same-sem increments (async DMA completions)
                    sync.wait_ge(out_sem, qt * 16)
                sync.dma_start(
                    out=out[qt * 128 : (qt + 1) * 128, :], in_=ots[qt % 3][:]
                ).then_inc(out_sem, 16)

        @block.tensor
        def _(tensor: bass.BassEngine):
            tensor.wait_ge(dma_in, 16)
            for qt in range(NQ):
                lhsT_hi = ops_sb[:, HI + L + qt * 128 : HI + L + (qt + 1) * 128]
                lhsT_lo = ops_sb[:, LO + L + qt * 128 : LO + L + (qt + 1) * 128]
                for j in range(NJ):
                    i = qt * NJ + j
                    if i >= 8:
                        # psum bank reuse: wait for the exp that drained it
                        tensor.wait_ge(act_sem, i - 7)
                    ps = banks[i % 8]
                    rhs_hi = ops_sb[:, HI + j * NB : HI + (j + 1) * NB]
                    rhs_lo = ops_sb[:, LO + j * NB : LO + (j + 1) * NB]
                    nc.tensor.matmul(ps, lhsT_hi, rhs_hi, start=True, stop=False)
                    nc.tensor.matmul(ps, lhsT_hi, rhs_lo, start=False, stop=False)
                    nc.tensor.matmul(
                        ps, lhsT_lo, rhs_hi, start=False, stop=True
                    ).then_inc(pe_sem, 1)

        @block.scalar
        def _(scalar: bass.BassEngine):
            for qt in range(NQ):
                if qt >= 3:
                    # output buffer reuse: wait for qt-3's store to finish
                    scalar.wait_ge(out_sem, (qt - 2) * 16)
                for j in range(NJ):
                    i = qt * NJ + j
                    scalar.wait_ge(pe_sem, i + 1)
                    nc.scalar.activation(
                        ots[qt % 3][:, j * NB : (j + 1) * NB],
                        banks[i % 8],
                        AF.Exp,
                    ).then_inc(act_sem, 1)

    _prog_cache["nc"] = nc
    return nc


def _bf16_split(a):
    hi = a.astype(ml_dtypes.bfloat16)
    lo = (a - hi.astype(np.float32)).astype(ml_dtypes.bfloat16)
    return hi, lo


def _make_in_maps(pos_embed, W_pos, b_pos, lengthscale, outputscale):
    ls = np.float32(np.asarray(lengthscale))
    os_ = np.float32(np.asarray(outputscale))
    ln_os = np.float64(np.log(np.float64(os_)))

    pe = np.asarray(pos_embed, dtype=np.float32)
    wls = (np.asarray(W_pos, dtype=np.float32) / ls).astype(np.float32)
    bls = (np.asarray(b_pos, dtype=np.float32) / ls).astype(np.float32)

    in_maps = []
    for c in range(NCORES):
        b = c // 2
        q0 = (c % 2) * QH
        xs = (wls @ pe[b] + bls[:, None]).astype(np.float32)  # [D, L]
        xs_hi, xs_lo = _bf16_split(xs)
        # effective squared norm as the PE computes the dot product:
        # (hi+lo).(hi+lo) - lo.lo = hi.hi + 2*hi.lo. Using it for the
        # -0.5||x||^2 rows makes the diagonal of the exponent exactly 0;
        # the dropped lo.lo term then only perturbs entries where exp()
        # is negligible (0.5*||lo_q - lo_k||^2, positive, ~1e-3).
        h64 = xs_hi.astype(np.float64)
        l64 = xs_lo.astype(np.float64)
        sq = (h64 * h64).sum(axis=0) + 2.0 * (h64 * l64).sum(axis=0)  # [L]

        v = -0.5 * sq + ln_os  # key-side row, ~-66
        w = -0.5 * sq[q0 : q0 + QH]  # query-side row
        v1 = v.astype(ml_dtypes.bfloat16)
        v2 = (v - v1.astype(np.float64)).astype(ml_dtypes.bfloat16)
        v3 = (v - v1.astype(np.float64) - v2.astype(np.float64)).astype(
            ml_dtypes.bfloat16
        )
        w1 = w.astype(ml_dtypes.bfloat16)
        w2 = (w - w1.astype(np.float64)).astype(ml_dtypes.bfloat16)
        w3 = (w - w1.astype(np.float64) - w2.astype(np.float64)).astype(
            ml_dtypes.bfloat16
        )

        one = ml_dtypes.bfloat16(1.0)
        zero = ml_dtypes.bfloat16(0.0)
        opsb = np.zeros((KQ, 2 * OPS_W), dtype=ml_dtypes.bfloat16)
        HI, LO = 0, OPS_W
        # keys (rhs), hi then lo
        opsb[0:D, HI : HI + L] = xs_hi
        opsb[D, HI : HI + L] = v1
        opsb[D + 1, HI : HI + L] = one
        opsb[D + 2, HI : HI + L] = v3
        opsb[D + 3, HI : HI + L] = one
        opsb[0:D, LO : LO + L] = xs_lo
        opsb[D, LO : LO + L] = v2
        # rows D+1..D+3 of lo-keys stay zero
        # queries (lhsT), hi then lo
        qs = slice(q0, q0 + QH)
        opsb[0:D, HI + L : HI + OPS_W] = xs_hi[:, qs]
        opsb[D, HI + L : HI + OPS_W] = one
        opsb[D + 1, HI + L : HI + OPS_W] = w1
        opsb[D + 2, HI + L : HI + OPS_W] = one
        opsb[D + 3, HI + L : HI + OPS_W] = w3
        opsb[0:D, LO + L : LO + OPS_W] = xs_lo[:, qs]
        opsb[D + 1, LO + L : LO + OPS_W] = w2
        # rows D, D+2, D+3 of lo-queries stay zero
        del zero
        in_maps.append({"ops": opsb})
    return in_maps


def run(pos_embed, W_pos, b_pos, lengthscale, outputscale, trace=False, **kw):
    """Run the kernel; returns (output [B, L, L] f32, BassKernelResults)."""
    nc = _build_program()
    in_maps = _make_in_maps(pos_embed, W_pos, b_pos, lengthscale, outputscale)
    res = run_bass_kernel_spmd(nc, in_maps, list(range(NCORES)), trace=trace, **kw)
    os_ = np.float32(np.asarray(outputscale))
    out = np.empty((B, L, L), dtype=np.float32)
    for c in range(NCORES):
        b = c // 2
        q0 = (c % 2) * QH
        # gpytorch clamps sq_dist at 0 => out <= outputscale
        np.minimum(res.results[c]["out"], os_, out=out[b, q0 : q0 + QH, :])
    return out, res


def kernel(pos_embed, W_pos, b_pos, lengthscale, outputscale):
    out, _ = run(pos_embed, W_pos, b_pos, lengthscale, outputscale)
    return out
